# revision 7
# baseline (speedup 1.0000x reference)
"""Box filter (radius 8, window 17, zero-padded edges) over dims 2,3 of a
[8, 32, 512, 512] f32 tensor, on 8 Trainium2 NeuronCores.

v4 (fp16 device pipeline, no-halo tiling, DVE/PE hybrid):
  - Harness tolerance is rel_err < 2e-2; computing on-device in fp16 (input
    quantized on host, output upconverted on host) halves HBM traffic.
    Measured numeric error ~5e-4 (scan state is fp32 internally).
  - Scan channels: column filter is ONE fused DVE `tensor_tensor_scan` per
    channel over a [128, 4*537] buffer holding four 128-row blocks padded
    [17 zeros | 512 data | 8 zeros]; the recurrence
        state[t] = (x[t] + state[t-1]) - x[t-17]
    flushes in the 25-zero inter-block gaps, so position 537*b + c + 8
    holds the window centered at column c of block b.  Row filter: main
    banded matmul per 128-row block + corner matmuls for the <=8 boundary
    rows from adjacent blocks, accumulated in PSUM.
  - The DVE scan is the bottleneck engine (~4.5us/channel, no 16-bit perf
    mode exists for scan), so X-path channels bypass DVE entirely: row
    filter via banded matmuls on the raw input, PSUM->SBUF copy, 16x
    128x128 fp16 DMA(xbar)-transposes, then the column filter as the same
    banded matmuls over column blocks.  Their output is stored transposed
    and fixed up on the host.  X second passes are deferred 4 channels so
    the in-order PE queue never waits on transposes.
  - GPSIMD does only the (tiny, strided) pad zeroing: measured HW fact --
    concurrent Pool-engine work slows DVE scans up to 2.5x (SBUF port
    contention), so no compute is offloaded there.

Sharding: data-parallel over batch (dim 0) -> 8 cores, one batch each.
"""

import os
import sys

import numpy as np

for _p in ("/opt/trn_rl_repo", "/root/.axon_site/_ro/trn_rl_repo"):
    if os.path.isdir(_p) and _p not in sys.path:
        sys.path.append(_p)

import concourse.bass as bass
import concourse.tile as tile
from concourse import bacc, mybir
from concourse.bass_utils import run_bass_kernel_spmd

R = 8
PADF = 2 * R + 1  # front zero pad per block (window width)
PADB = R          # back zero pad per block
H = W = 512
CH = 32
NCORES = 8
NB = 4            # 128-row blocks per channel
XW = PADF + W + PADB          # 537 block stride in the scan buffer
XALL = NB * XW                # 2148
UBW = XALL - PADF             # 2131 scan output width

# X-path channels (column filter on PE instead of DVE)
X_N = int(os.environ.get("BOX_XPATH", "6"))
X_SET = frozenset(2 + 5 * i for i in range(X_N))
X_DELAY = 4  # channels between an X first pass and its second pass

_CACHE = {}


def _banded():
    k = np.arange(128)[:, None]
    m = np.arange(128)[None, :]
    # main: block t rows -> tile t outputs, |k - m| <= 8
    bm = (np.abs(k - m) <= R).astype(np.float16)
    # prev corner: block t-1 row k -> output m: k >= m + 120
    cp = ((k >= m + 120) & (m <= 7)).astype(np.float16)
    # next corner: block t+1 row k -> output m: k <= m - 120
    cn = ((k <= m - 120) & (m >= 120)).astype(np.float16)
    return bm, cp, cn


def _build_program():
    if "nc" in _CACHE:
        return _CACHE["nc"]
    nc = bacc.Bacc(debug=False)
    f16 = mybir.dt.float16
    f32 = mybir.dt.float32
    x = nc.dram_tensor("x", [CH, H, W], f16, kind="ExternalInput")
    z = nc.dram_tensor("z", [CH, H, W], f16, kind="ExternalOutput")
    bm = nc.dram_tensor("bm", [128, 128], f16, kind="ExternalInput")
    cp = nc.dram_tensor("cp", [128, 128], f16, kind="ExternalInput")
    cn = nc.dram_tensor("cn", [128, 128], f16, kind="ExternalInput")
    xap, zap = x.ap(), z.ap()

    NBIG = 6   # xa ring
    NUB = 4    # scan-out ring
    NOG = 4    # output ring
    NXR = 2    # X-path Y / YT / out rings

    with tile.TileContext(nc) as tc:
        with (
            tc.tile_pool(name="consts", bufs=1) as cpool,
            tc.tile_pool(name="psum", bufs=2, space="PSUM") as ppool,
        ):
            bmt = cpool.tile([128, 128], f16)
            cpt = cpool.tile([128, 128], f16)
            cnt = cpool.tile([128, 128], f16)

            xas = [
                nc.alloc_sbuf_tensor(f"xa{i}", [128, XALL], f16).ap()
                for i in range(NBIG)
            ]
            ubs = [
                nc.alloc_sbuf_tensor(f"ub{i}", [128, UBW], f16).ap()
                for i in range(NUB)
            ]
            ogs = [
                nc.alloc_sbuf_tensor(f"og{i}", [128, NB, W], f16).ap()
                for i in range(NOG)
            ]
            ys = [
                nc.alloc_sbuf_tensor(f"y{i}", [128, NB, W], f16).ap()
                for i in range(NXR)
            ]
            yts = [
                nc.alloc_sbuf_tensor(f"yt{i}", [128, NB, W], f16).ap()
                for i in range(NXR)
            ]
            ogxs = [
                nc.alloc_sbuf_tensor(f"ogx{i}", [128, NB, W], f16).ap()
                for i in range(NXR)
            ]

            def _zero_pads(xb):
                # loads only ever write the data columns, so zeroing just
                # the pad columns once is enough for the whole kernel
                front = bass.AP(
                    tensor=xb.tensor, offset=xb.offset,
                    ap=[[XALL, 128], [XW, NB], [1, PADF]],
                )
                back = bass.AP(
                    tensor=xb.tensor, offset=xb.offset + PADF + W,
                    ap=[[XALL, 128], [XW, NB], [1, PADB]],
                )
                nc.gpsimd.memset(front, 0.0)
                nc.gpsimd.memset(back, 0.0)

            _zero_pads(xas[0])

            def band_mms(ps, rhs_of_block):
                """main + corner banded matmuls for 4 output tiles into the
                4-bank psum tile; rhs_of_block(b) -> [128, W] AP."""
                for t in range(NB):
                    nc.tensor.matmul(
                        ps[:, t, :], bmt[0:128, 0:128], rhs_of_block(t),
                        start=True, stop=False,
                    )
                    if t > 0:
                        nc.tensor.matmul(
                            ps[:, t, :], cpt[0:128, 0:128],
                            rhs_of_block(t - 1),
                            start=False, stop=(t == NB - 1),
                        )
                    if t < NB - 1:
                        nc.tensor.matmul(
                            ps[:, t, :], cnt[0:128, 0:128],
                            rhs_of_block(t + 1),
                            start=False, stop=True,
                        )

            xlist = sorted(X_SET)

            def second_pass(cx):
                """X-path: column filter over YT's column blocks; stores the
                transposed result (host swaps it back)."""
                i = xlist.index(cx)
                yt = yts[i % NXR]
                ogx = ogxs[i % NXR]
                ps = ppool.tile([128, NB, W], f32)
                band_mms(ps, lambda b: yt[:, b, :])
                nc.scalar.copy(ogx[:, :, :], ps[:, :, :])
                nc.scalar.dma_start(
                    zap[cx, :, :].rearrange("(t p) w -> p t w", p=128),
                    ogx[:, :, :],
                )

            for c in range(CH):
                if c - X_DELAY in X_SET:
                    second_pass(c - X_DELAY)

                xa = xas[c % NBIG]

                # one 512 KB load: (p, b, col) <- x[c, 128b + p, col]
                src = xap[c, :, :].rearrange("(b p) w -> p b w", p=128)
                dst = bass.AP(
                    tensor=xa.tensor,
                    offset=xa.offset + PADF,
                    ap=[[XALL, 128], [XW, NB], [1, W]],
                )
                nc.sync.dma_start(dst, src)
                if c == 0:
                    # consts + remaining pad zeroing overlap channel 0's load
                    nc.sync.dma_start(bmt[:], bm.ap()[:, :])
                    nc.sync.dma_start(cpt[:], cp.ap()[:, :])
                    nc.sync.dma_start(cnt[:], cn.ap()[:, :])
                    for xb in xas[1:]:
                        _zero_pads(xb)

                if c in X_SET:
                    i = xlist.index(c)
                    y = ys[i % NXR]
                    yt = yts[i % NXR]
                    ps = ppool.tile([128, NB, W], f32)
                    # row filter directly on the raw input blocks
                    band_mms(
                        ps,
                        lambda b: xa[:, b * XW + PADF:b * XW + PADF + W],
                    )
                    nc.scalar.copy(y[:, :, :], ps[:, :, :])
                    # 16x 128x128 xbar transposes:
                    # yt[pc, bc, 128*br + r] = y[r, br, 128*bc + pc]
                    for br in range(NB):
                        for bc in range(NB):
                            nc.sync.dma_start_transpose(
                                yt[:, bc, br * 128:(br + 1) * 128],
                                y[:, br, bc * 128:(bc + 1) * 128],
                            )
                    continue

                ub = ubs[c % NUB]
                og = ogs[c % NOG]

                # one scan covers all 4 blocks (recurrence flushes in the
                # 25-zero inter-block gaps).  out[t] = window of 17 ending
                # at data0 position t.
                nc.vector.tensor_tensor_scan(
                    out=ub[:, 0:UBW],
                    data0=xa[:, PADF:XALL],
                    data1=xa[:, 0:UBW],
                    initial=0.0,
                    op0=mybir.AluOpType.add,
                    op1=mybir.AluOpType.subtract,
                )

                ps = ppool.tile([128, NB, W], f32)
                band_mms(ps, lambda b: ub[:, b * XW + R:b * XW + R + W])
                # one big PSUM->SBUF copy (amortizes PSUM access latency
                # over all 4 banks), one 512 KB store
                nc.scalar.copy(og[:, :, :], ps[:, :, :])
                nc.scalar.dma_start(
                    zap[c, :, :].rearrange("(t p) w -> p t w", p=128),
                    og[:, :, :],
                )

            for cx in xlist:
                if cx + X_DELAY >= CH:
                    second_pass(cx)

    nc.compile()
    _CACHE["nc"] = nc
    return nc


def _fix_layout(zc: np.ndarray) -> np.ndarray:
    """X-path channels are stored transposed; swap them back. [CH, H, W]."""
    if X_SET:
        idx = sorted(X_SET)
        zc[idx] = np.swapaxes(zc[idx], -2, -1)
    return zc


def kernel(tensor: np.ndarray) -> np.ndarray:
    tensor = np.asarray(tensor)
    assert tensor.shape == (NCORES, CH, H, W)
    x16 = tensor.astype(np.float16)
    bm, cp, cn = _banded()
    nc = _build_program()
    in_maps = [
        {"x": x16[i], "bm": bm, "cp": cp, "cn": cn} for i in range(NCORES)
    ]
    res = run_bass_kernel_spmd(nc, in_maps, core_ids=list(range(NCORES)))
    out = np.stack(
        [_fix_layout(np.array(res.results[i]["z"])) for i in range(NCORES)],
        axis=0,
    )
    return out.astype(np.float32)


# revision 8
# speedup vs baseline: 1.4957x; 1.4957x over previous
"""Box filter (radius 8, window 17, zero-padded edges) over dims 2,3 of a
[8, 32, 512, 512] f32 tensor, on 8 Trainium2 NeuronCores.

v4 (fp16 device pipeline, no-halo tiling, DVE/PE hybrid):
  - Harness tolerance is rel_err < 2e-2; computing on-device in fp16 (input
    quantized on host, output upconverted on host) halves HBM traffic.
    Measured numeric error ~5e-4 (scan state is fp32 internally).
  - Scan channels: column filter is ONE fused DVE `tensor_tensor_scan` per
    channel over a [128, 4*537] buffer holding four 128-row blocks padded
    [17 zeros | 512 data | 8 zeros]; the recurrence
        state[t] = (x[t] + state[t-1]) - x[t-17]
    flushes in the 25-zero inter-block gaps, so position 537*b + c + 8
    holds the window centered at column c of block b.  Row filter: main
    banded matmul per 128-row block + corner matmuls for the <=8 boundary
    rows from adjacent blocks, accumulated in PSUM.
  - The DVE scan is the bottleneck engine (~4.5us/channel, no 16-bit perf
    mode exists for scan), so X-path channels bypass DVE entirely: row
    filter via banded matmuls on the raw input, PSUM->SBUF copy, 16x
    128x128 fp16 DMA(xbar)-transposes, then the column filter as the same
    banded matmuls over column blocks.  Their output is stored transposed
    and fixed up on the host.  X second passes are deferred 4 channels so
    the in-order PE queue never waits on transposes.
  - GPSIMD does only the (tiny, strided) pad zeroing: measured HW fact --
    concurrent Pool-engine work slows DVE scans up to 2.5x (SBUF port
    contention), so no compute is offloaded there.

Sharding: data-parallel over batch (dim 0) -> 8 cores, one batch each.
"""

import os
import sys

import numpy as np

for _p in ("/opt/trn_rl_repo", "/root/.axon_site/_ro/trn_rl_repo"):
    if os.path.isdir(_p) and _p not in sys.path:
        sys.path.append(_p)

import concourse.bass as bass
import concourse.tile as tile
from concourse import bacc, mybir
from concourse.bass_utils import run_bass_kernel_spmd

R = 8
PADF = 2 * R + 1  # front zero pad per block (window width)
PADB = R          # back zero pad per block
H = W = 512
CH = 32
NCORES = 8
NB = 4            # 128-row blocks per channel
XW = PADF + W + PADB          # 537 block stride in the scan buffer
XALL = NB * XW                # 2148
UBW = XALL - PADF             # 2131 scan output width

# X-path channels (column filter on PE instead of DVE)
X_N = int(os.environ.get("BOX_XPATH", "6"))
X_SET = frozenset(2 + 5 * i for i in range(X_N))
X_DELAY = 4  # channels between an X first pass and its second pass

_CACHE = {}


def _banded():
    k = np.arange(128)[:, None]
    m = np.arange(128)[None, :]
    # main: block t rows -> tile t outputs, |k - m| <= 8
    bm = (np.abs(k - m) <= R).astype(np.float16)
    # prev corner: block t-1 row k -> output m: k >= m + 120
    cp = ((k >= m + 120) & (m <= 7)).astype(np.float16)
    # next corner: block t+1 row k -> output m: k <= m - 120
    cn = ((k <= m - 120) & (m >= 120)).astype(np.float16)
    return bm, cp, cn


def _build_program():
    if "nc" in _CACHE:
        return _CACHE["nc"]
    nc = bacc.Bacc(debug=False)
    f16 = mybir.dt.float16
    f32 = mybir.dt.float32
    x = nc.dram_tensor("x", [CH, H, W], f16, kind="ExternalInput")
    z = nc.dram_tensor("z", [CH, H, W], f16, kind="ExternalOutput")
    bm = nc.dram_tensor("bm", [128, 128], f16, kind="ExternalInput")
    cp = nc.dram_tensor("cp", [128, 128], f16, kind="ExternalInput")
    cn = nc.dram_tensor("cn", [128, 128], f16, kind="ExternalInput")
    xap, zap = x.ap(), z.ap()

    NBIG = 6   # xa ring
    NUB = 4    # scan-out ring
    NOG = 4    # output ring
    NXR = 2    # X-path Y / YT / out rings

    with tile.TileContext(nc) as tc:
        with (
            tc.tile_pool(name="consts", bufs=1) as cpool,
            tc.tile_pool(name="psum", bufs=2, space="PSUM") as ppool,
        ):
            bmt = cpool.tile([128, 128], f16)
            cpt = cpool.tile([128, 128], f16)
            cnt = cpool.tile([128, 128], f16)

            xas = [
                nc.alloc_sbuf_tensor(f"xa{i}", [128, XALL], f16).ap()
                for i in range(NBIG)
            ]
            ubs = [
                nc.alloc_sbuf_tensor(f"ub{i}", [128, UBW], f16).ap()
                for i in range(NUB)
            ]
            ogs = [
                nc.alloc_sbuf_tensor(f"og{i}", [128, NB, W], f16).ap()
                for i in range(NOG)
            ]
            ys = [
                nc.alloc_sbuf_tensor(f"y{i}", [128, NB, W], f16).ap()
                for i in range(NXR)
            ]
            # yt layout [128, 16, 128]: yt[p, m, r] = y[r, 128m+p]
            # (m = 4*br + bc); one xbar transpose fills it per X-channel
            yts = [
                nc.alloc_sbuf_tensor(f"yt{i}", [128, NB * W], f16).ap()
                for i in range(NXR)
            ]
            ogxs = [
                nc.alloc_sbuf_tensor(f"ogx{i}", [128, NB, W], f16).ap()
                for i in range(NXR)
            ]

            def _zero_pads(xb):
                # loads only ever write the data columns, so zeroing just
                # the pad columns once is enough for the whole kernel
                front = bass.AP(
                    tensor=xb.tensor, offset=xb.offset,
                    ap=[[XALL, 128], [XW, NB], [1, PADF]],
                )
                back = bass.AP(
                    tensor=xb.tensor, offset=xb.offset + PADF + W,
                    ap=[[XALL, 128], [XW, NB], [1, PADB]],
                )
                nc.gpsimd.memset(front, 0.0)
                nc.gpsimd.memset(back, 0.0)

            _zero_pads(xas[0])

            def band_mms(ps, rhs_of_block):
                """main + corner banded matmuls for 4 output tiles into the
                4-bank psum tile; rhs_of_block(b) -> [128, W] AP."""
                for t in range(NB):
                    nc.tensor.matmul(
                        ps[:, t, :], bmt[0:128, 0:128], rhs_of_block(t),
                        start=True, stop=False,
                    )
                    if t > 0:
                        nc.tensor.matmul(
                            ps[:, t, :], cpt[0:128, 0:128],
                            rhs_of_block(t - 1),
                            start=False, stop=(t == NB - 1),
                        )
                    if t < NB - 1:
                        nc.tensor.matmul(
                            ps[:, t, :], cnt[0:128, 0:128],
                            rhs_of_block(t + 1),
                            start=False, stop=True,
                        )

            xlist = sorted(X_SET)

            def second_pass(cx):
                """X-path: column filter over YT's column blocks; stores the
                transposed result (host swaps it back)."""
                i = xlist.index(cx)
                yt = yts[i % NXR]
                ogx = ogxs[i % NXR]
                ps = ppool.tile([128, NB, W], f32)

                def rhs_col_block(b):
                    # column block b of the transposed channel: rows
                    # enumerated (br, r) -> psum col 128*br + r
                    return bass.AP(
                        tensor=yt.tensor,
                        offset=yt.offset + b * 128,
                        ap=[[NB * W, 128], [4 * 128, NB], [1, 128]],
                    )

                band_mms(ps, rhs_col_block)
                nc.scalar.copy(ogx[:, :, :], ps[:, :, :])
                nc.scalar.dma_start(
                    zap[cx, :, :].rearrange("(t p) w -> p t w", p=128),
                    ogx[:, :, :],
                )

            for c in range(CH):
                if c - X_DELAY in X_SET:
                    second_pass(c - X_DELAY)

                xa = xas[c % NBIG]

                # one 512 KB load: (p, b, col) <- x[c, 128b + p, col]
                src = xap[c, :, :].rearrange("(b p) w -> p b w", p=128)
                dst = bass.AP(
                    tensor=xa.tensor,
                    offset=xa.offset + PADF,
                    ap=[[XALL, 128], [XW, NB], [1, W]],
                )
                nc.sync.dma_start(dst, src)
                if c == 0:
                    # consts + remaining pad zeroing overlap channel 0's load
                    nc.sync.dma_start(bmt[:], bm.ap()[:, :])
                    nc.sync.dma_start(cpt[:], cp.ap()[:, :])
                    nc.sync.dma_start(cnt[:], cn.ap()[:, :])
                    for xb in xas[1:]:
                        _zero_pads(xb)

                if c in X_SET:
                    i = xlist.index(c)
                    y = ys[i % NXR]
                    yt = yts[i % NXR]
                    ps = ppool.tile([128, NB, W], f32)
                    # row filter directly on the raw input blocks
                    band_mms(
                        ps,
                        lambda b: xa[:, b * XW + PADF:b * XW + PADF + W],
                    )
                    nc.scalar.copy(y[:, :, :], ps[:, :, :])
                    # ONE xbar transpose for the whole channel:
                    # yt[p, m, r] = y_flat[r, 128m + p]
                    yt3d = bass.AP(
                        tensor=yt.tensor,
                        offset=yt.offset,
                        ap=[[NB * W, 128], [128, 16], [1, 128]],
                    )
                    nc.sync.dma_start_transpose(yt3d, y[:, :, :])
                    continue

                ub = ubs[c % NUB]
                og = ogs[c % NOG]

                # one scan covers all 4 blocks (recurrence flushes in the
                # 25-zero inter-block gaps).  out[t] = window of 17 ending
                # at data0 position t.
                nc.vector.tensor_tensor_scan(
                    out=ub[:, 0:UBW],
                    data0=xa[:, PADF:XALL],
                    data1=xa[:, 0:UBW],
                    initial=0.0,
                    op0=mybir.AluOpType.add,
                    op1=mybir.AluOpType.subtract,
                )

                ps = ppool.tile([128, NB, W], f32)
                band_mms(ps, lambda b: ub[:, b * XW + R:b * XW + R + W])
                # one big PSUM->SBUF copy (amortizes PSUM access latency
                # over all 4 banks), one 512 KB store
                nc.scalar.copy(og[:, :, :], ps[:, :, :])
                nc.scalar.dma_start(
                    zap[c, :, :].rearrange("(t p) w -> p t w", p=128),
                    og[:, :, :],
                )

            for cx in xlist:
                if cx + X_DELAY >= CH:
                    second_pass(cx)

    nc.compile()
    _CACHE["nc"] = nc
    return nc


def _fix_layout(zc: np.ndarray) -> np.ndarray:
    """X-path channels are stored transposed; swap them back. [CH, H, W]."""
    if X_SET:
        idx = sorted(X_SET)
        zc[idx] = np.swapaxes(zc[idx], -2, -1)
    return zc


def kernel(tensor: np.ndarray) -> np.ndarray:
    tensor = np.asarray(tensor)
    assert tensor.shape == (NCORES, CH, H, W)
    x16 = tensor.astype(np.float16)
    bm, cp, cn = _banded()
    nc = _build_program()
    in_maps = [
        {"x": x16[i], "bm": bm, "cp": cp, "cn": cn} for i in range(NCORES)
    ]
    res = run_bass_kernel_spmd(nc, in_maps, core_ids=list(range(NCORES)))
    out = np.stack(
        [_fix_layout(np.array(res.results[i]["z"])) for i in range(NCORES)],
        axis=0,
    )
    return out.astype(np.float32)
